# revision 51
# baseline (speedup 1.0000x reference)
"""Trainium2 Bass kernel: segment-mean over token segments + pairwise-diff edge MLP.

Reference computation (per batch row b):
  seg = cumsum(ids == 3); valid = ids != 3
  means[n] = mean of features[s] over tokens with seg==n & valid (n < 8), 0-count -> sum/1
  diff[i,j] = means[i] - means[j]                          # [8,8,H]
  out[i,j]  = relu(relu(diff @ W1 + b1) @ Wm + bm) @ W2 + b2   # [8,8,150]

Distribution: data-parallel over batch B=128 across 8 NeuronCores (16 rows/core),
tiny MLP weights replicated, no cross-core communication.

v2 design (vs baseline):
  - All streamed data is bf16 (host-side cast): halves HBM traffic, which is
    the bottleneck. Accumulation stays fp32 in PSUM. rel-err budget 2e-2
    absorbs bf16 rounding (~1e-3) easily.
  - Features are host-packed to the SBUF layout [128 tok-part, row, t, H] so
    each DMA reads 6KB-contiguous per partition line; one DMA per (row,
    half-of-t) so the first matmul can start after ~1/32 of the stream.
  - The pairwise-diff + mm1 matmul pair is replaced via linearity:
    W1^T (m_i - m_j) = u_i - u_j with u = W1^T m. means are transposed with
    XBAR dma_start_transpose (tiny), u is a 32-col matmul, and the 64 pair
    differences come from one broadcast DVE subtract per c-chunk. This cuts
    PE work by ~25% and removes the e4 trick entirely.
  - mm3 moving dim is 150 (bf16 runs full rate without the >=256 pad fp32r
    needed).
Stage-1 PSUM evictions (x 1/count, cast to bf16) run on the Pool engine,
relu+bias activations on Scalar, pair-diff + output copies on DVE, so no
single helper engine becomes the new bottleneck.
"""

import sys

import numpy as np

if "/opt/trn_rl_repo" not in sys.path:
    sys.path.insert(0, "/opt/trn_rl_repo")

import ml_dtypes

import concourse.bass as bass
import concourse.mybir as mybir
from concourse.bass import ds
from concourse.bass_utils import run_bass_kernel_spmd
from concourse.tile import TileContext

B, S, H, C = 128, 1024, 768, 150
NSEG = 8
SEP_ID = 3
NCORES = 8
RPC = B // NCORES  # 16 rows per core
TCH = S // 128     # 8 token chunks
HC = H // 128      # 6 hidden chunks
HHALF = 384        # H split for PSUM bank limit
CC = ((0, 128), (128, 22))  # c-dim (150) chunks

F32 = mybir.dt.float32
BF16 = mybir.dt.bfloat16
NPBF16 = ml_dtypes.bfloat16


def build_program(rpc=RPC, tch=TCH, feat_bufs=12):
    ngp = rpc // 4  # groups of 4 batch rows -> 256 (r4,i,j) output rows each
    nc = bass.Bass("TRN2", target_bir_lowering=False, debug=False)

    feats_d = nc.dram_tensor("features", [128, rpc * tch * H], BF16,
                             kind="ExternalInput").ap()
    ohT_d = nc.dram_tensor("ohT", [128, rpc * tch * NSEG], BF16,
                           kind="ExternalInput").ap()
    icnt_d = nc.dram_tensor("icnt", [NSEG, rpc], F32, kind="ExternalInput").ap()
    wbig_d = nc.dram_tensor("wbig", [128, (HC + 2) * C], BF16,
                            kind="ExternalInput").ap()
    wsml_d = nc.dram_tensor("wsml", [22, 2 * C], BF16,
                            kind="ExternalInput").ap()
    bb_d = nc.dram_tensor("bb", [128, 2], F32, kind="ExternalInput").ap()
    bs_d = nc.dram_tensor("bs", [22, 2], F32, kind="ExternalInput").ap()
    b2p_d = nc.dram_tensor("b2p", [1, C], BF16, kind="ExternalInput").ap()
    ones_d = nc.dram_tensor("ones", [1, 128], BF16, kind="ExternalInput").ap()
    eye8_d = nc.dram_tensor("eye8", [NSEG, NSEG], BF16, kind="ExternalInput").ap()
    out_d = nc.dram_tensor("out", [ngp * 256, C], BF16,
                           kind="ExternalOutput").ap()

    RELU = mybir.ActivationFunctionType.Relu
    COPY = mybir.ActivationFunctionType.Copy
    MULT = mybir.AluOpType.mult
    SUB = mybir.AluOpType.subtract
    ADD = mybir.AluOpType.add
    MAX = mybir.AluOpType.max

    with TileContext(nc) as tc:
        with (
            tc.tile_pool(name="const", bufs=1) as constp,
            tc.tile_pool(name="featp", bufs=feat_bufs) as featp,
            tc.tile_pool(name="meansp", bufs=8) as meansp,
            tc.tile_pool(name="mtp", bufs=2) as mtp,
            tc.tile_pool(name="usbp", bufs=4) as usbp,
            tc.tile_pool(name="dup", bufs=2) as dup,
            tc.tile_pool(name="h1p", bufs=3) as h1p,
            tc.tile_pool(name="h2p", bufs=3) as h2p,
            tc.tile_pool(name="osbp", bufs=3) as osbp,
            tc.tile_pool(name="mpsum", bufs=2, space="PSUM") as mpsum,
            tc.tile_pool(name="tpsum", bufs=2, space="PSUM") as tpsum,
            tc.tile_pool(name="spsum", bufs=2, space="PSUM") as spsum,
        ):
            # Each HWDGE dma_start costs its sequencer ~600-800ns (DIRECT2D
            # descriptor gen), so only the one const that gates the first
            # matmul (the first rows' one-hots) goes ahead of the feature
            # quarters on the sync ring. Everything else rides the scalar
            # HWDGE ring (stripes over all 16 DMA engines) or gpsimd SWDGE.
            ohT_sb = constp.tile([128, rpc * tch * NSEG], BF16, tag="c_ohT")
            nc.sync.dma_start(out=ohT_sb[:, ds(0, 2 * tch * NSEG)],
                              in_=ohT_d[:, ds(0, 2 * tch * NSEG)])
            eye8_sb = constp.tile([NSEG, NSEG], BF16, tag="c_eye8")
            nc.gpsimd.dma_start(out=eye8_sb, in_=eye8_d)
            icnt_sb = constp.tile([NSEG, rpc], F32, tag="c_icnt")
            nc.gpsimd.dma_start(out=icnt_sb, in_=icnt_d)
            # rest of the one-hots (needed from row 2, ~19us in)
            nc.scalar.dma_start(
                out=ohT_sb[:, ds(2 * tch * NSEG, (rpc - 2) * tch * NSEG)],
                in_=ohT_d[:, ds(2 * tch * NSEG, (rpc - 2) * tch * NSEG)])
            # weights+biases packed host-side (all bf16) -> 4 triggers
            # instead of 13 (each trigger costs its sequencer ~700ns of
            # DIRECT2D descriptor generation)
            wbig_sb = constp.tile([128, (HC + 2) * C], BF16, tag="c_wbig")
            nc.scalar.dma_start(out=wbig_sb, in_=wbig_d)
            wsml_sb = constp.tile([22, 2 * C], BF16, tag="c_wsml")
            nc.scalar.dma_start(out=wsml_sb, in_=wsml_d)
            bb_sb = constp.tile([128, 2], F32, tag="c_bb")
            nc.scalar.dma_start(out=bb_sb, in_=bb_d)
            bs_sb = constp.tile([22, 2], F32, tag="c_bs")
            nc.scalar.dma_start(out=bs_sb, in_=bs_d)
            b2p_sb = constp.tile([1, C], BF16, tag="c_b2")
            nc.scalar.dma_start(out=b2p_sb, in_=b2p_d)
            ones_sb = constp.tile([1, 128], BF16, tag="c_ones")
            nc.scalar.dma_start(out=ones_sb, in_=ones_d)
            w1_sb = wbig_sb                      # [:, hc*C+coff] slices
            wm0_sb = wbig_sb[:, ds(HC * C, C)]
            w20_sb = wbig_sb[:, ds((HC + 1) * C, C)]
            wm1_sb = wsml_sb[:, ds(0, C)]
            w21_sb = wsml_sb[:, ds(C, C)]
            b1_sb = [bb_sb[:, ds(0, 1)], bs_sb[:, ds(0, 1)]]
            bm_sb = [bb_sb[:, ds(1, 1)], bs_sb[:, ds(1, 1)]]

            # warm-up matmuls: keep the PE executing from t~0 so the clock
            # has ramped to the fast pstate when the first features land
            # (junk math on the first ohT slice; result never read)
            warm = mpsum.tile([NSEG, NSEG], F32, tag="mpA")
            for _ in range(40):
                nc.tensor.matmul(warm, ohT_sb[:, ds(0, NSEG)],
                                 ohT_sb[:, ds(0, NSEG)], start=True, stop=True)

            for gp in range(ngp):
                # ---- stage 1: segment means for 4 batch rows ----
                tps = tpsum.tile([128, HC, 4 * NSEG], BF16, tag="tps")
                prev = None
                for r4 in range(4):
                    row = gp * 4 + r4
                    # all feature DMAs ride the sync ring, which carries no
                    # compute ops -> no head-of-line blocking on PE/PSUM
                    # waits. rows 0-1 are quartered so the first matmul can
                    # start early; later rows use bigger half-row DMAs.
                    if row < 2:
                        nch, tpc, tag = 4, 2, "featq"
                    else:
                        nch, tpc, tag = 2, 4, "feat"
                    fq = []
                    for th in range(nch):
                        ft = featp.tile([128, tpc * H], BF16, tag=tag)
                        nc.sync.dma_start(
                            out=ft,
                            in_=feats_d[:, ds((row * tch + th * tpc) * H,
                                              tpc * H)],
                        )
                        fq.append(ft)
                    mpA = mpsum.tile([NSEG, HHALF], F32, tag="mpA")
                    mpB = mpsum.tile([NSEG, HHALF], F32, tag="mpB")
                    for t in range(tch):
                        f = fq[t // tpc]
                        toff = (t % tpc) * H
                        ohs = ohT_sb[:, ds((row * tch + t) * NSEG, NSEG)]
                        nc.tensor.matmul(
                            mpA, ohs, f[:, ds(toff, HHALF)],
                            start=(t == 0), stop=(t == tch - 1),
                        )
                        nc.tensor.matmul(
                            mpB, ohs, f[:, ds(toff + HHALF, HHALF)],
                            start=(t == 0), stop=(t == tch - 1),
                        )
                    # evict x (1/count), fp32 -> bf16 (GpSimd cannot read
                    # PSUM on TRN2 -> split across Scalar and Vector)
                    m = meansp.tile([NSEG, H], BF16, tag="means")
                    icol = icnt_sb[:, ds(row, 1)]
                    nc.scalar.activation(m[:, ds(0, HHALF)], mpA, COPY,
                                         scale=icol)
                    nc.vector.tensor_scalar_mul(m[:, ds(HHALF, HHALF)], mpB,
                                                icol)
                    # transpose the PREVIOUS row's means now: its evictions
                    # completed while this row's matmuls ran, so the PE
                    # stream never blocks on the scalar/vector engines
                    if prev is not None:
                        pm, pr = prev
                        for hc in range(HC):
                            nc.tensor.transpose(
                                tps[:, hc, ds(pr * NSEG, NSEG)],
                                pm[:, ds(hc * 128, 128)],
                                eye8_sb,
                            )
                    prev = (m, r4)

                pm, pr = prev
                for hc in range(HC):
                    nc.tensor.transpose(
                        tps[:, hc, ds(pr * NSEG, NSEG)],
                        pm[:, ds(hc * 128, 128)],
                        eye8_sb,
                    )
                mT = mtp.tile([128, HC, 4 * NSEG], BF16, tag="mT")
                nc.vector.tensor_copy(mT, tps)

                # ---- u = W1^T @ mT : [c, (r4, seg)] ----
                u_sb = []
                for ci, (coff, csz) in enumerate(CC):
                    ups = spsum.tile([csz, 4 * NSEG], F32, tag="sp")
                    for hc in range(HC):
                        nc.tensor.matmul(
                            ups, w1_sb[:, ds(hc * C + coff, csz)], mT[:, hc, :],
                            start=(hc == 0), stop=(hc == HC - 1),
                        )
                    us = usbp.tile([csz, 4 * NSEG], F32, tag=f"u{ci}")
                    nc.scalar.activation(us, ups, COPY)
                    u_sb.append(us)

                # ---- pairwise diff via broadcast DVE sub + relu(+b1) ----
                h1 = []
                for ci, (coff, csz) in enumerate(CC):
                    us = u_sb[ci]
                    du = dup.tile([csz, 256], F32, tag=f"du{ci}")
                    for r4 in range(4):
                        sl = us[:, ds(r4 * NSEG, NSEG)]
                        ap_i = bass.AP(sl.tensor, sl.offset,
                                       [sl.ap[0], [1, NSEG], [0, NSEG]])
                        ap_j = bass.AP(sl.tensor, sl.offset,
                                       [sl.ap[0], [0, NSEG], [1, NSEG]])
                        dv = du[:, ds(r4 * 64, 64)].rearrange(
                            "p (i j) -> p i j", i=NSEG, j=NSEG)
                        nc.vector.scalar_tensor_tensor(
                            dv, ap_i, 1.0, ap_j, MULT, SUB)
                    hs = h1p.tile([csz, 256], BF16, tag=f"h1s{ci}")
                    # relu(du + b1) on DVE: (du add b1) max 0 — keeps the
                    # scalar queue free so stage-1 evictions stay prompt
                    nc.vector.tensor_scalar(hs, du, b1_sb[ci], 0.0, ADD, MAX)
                    h1.append(hs)

                # ---- mm2: h2T = relu(Wm^T @ h1T + bm) ----
                h2 = []
                for ci, (coff, csz) in enumerate(CC):
                    hp = spsum.tile([csz, 256], F32, tag="sp")
                    nc.tensor.matmul(hp, wbig_sb[:, ds(HC * C + coff, csz)],
                                     h1[0], start=True, stop=False)
                    nc.tensor.matmul(hp, wsml_sb[:, ds(coff, csz)],
                                     h1[1], start=False, stop=True)
                    hs = h2p.tile([csz, 256], BF16, tag=f"h2s{ci}")
                    nc.scalar.activation(hs, hp, RELU, bias=bm_sb[ci])
                    h2.append(hs)

                # ---- mm3: out = h2 @ W2 + b2, natural [rows, c] layout ----
                osb = osbp.tile([128, 2, C], BF16, tag="osb")
                for rs in range(2):
                    op = spsum.tile([128, C], F32, tag="sp")
                    nc.tensor.matmul(op, h2[0][:, ds(rs * 128, 128)],
                                     w20_sb, start=True, stop=False)
                    nc.tensor.matmul(op, h2[1][:, ds(rs * 128, 128)],
                                     w21_sb, start=False, stop=False)
                    nc.tensor.matmul(op, ones_sb,
                                     b2p_sb, start=False, stop=True)
                    nc.vector.tensor_copy(osb[:, rs, :], op)
                nc.scalar.dma_start(
                    out=out_d[ds(gp * 256, 256), :].rearrange(
                        "(rs p) c -> p rs c", p=128),
                    in_=osb,
                )

    # TRN2 allows at most 1 sync wait per instruction (2 on event semaphores).
    # Tile can emit more; split them the same way Bacc.compile() does.
    import bass_rust as _bass_rust
    _bass_rust.move_matmul_waits_to_ldweights(nc.m)
    _bass_rust.generate_event_semaphores(nc)
    return nc


def host_prep(output_ids, features, W1, b1, Wm, bm, W2, b2, rpc=RPC, tch=TCH):
    """Build per-core input maps. features/one-hot are repacked to the device
    SBUF layout and cast to bf16 host-side (halves HBM traffic)."""
    ids = np.asarray(output_ids)
    nrows = ids.shape[0]
    ncores = nrows // rpc
    feats = np.asarray(features)

    is_sep = ids == SEP_ID
    seg = np.cumsum(is_sep.astype(np.int64), axis=1)
    valid = ~is_sep
    oh = ((seg[:, :, None] == np.arange(NSEG)[None, None, :]) & valid[:, :, None])
    counts = oh.sum(axis=1)                           # [B, 8]
    icnt_full = (1.0 / np.maximum(counts, 1.0)).astype(np.float32)
    oh16 = oh.astype(NPBF16)                          # [B, S, 8] exact {0,1}

    W1 = np.asarray(W1, np.float32)
    Wm = np.asarray(Wm, np.float32)
    W2 = np.asarray(W2, np.float32)
    b1 = np.asarray(b1, np.float32)
    bm = np.asarray(bm, np.float32)
    b2 = np.asarray(b2, np.float32)

    w1p = W1.reshape(HC, 128, C).transpose(1, 0, 2).reshape(128, HC * C)
    wbig = np.concatenate([w1p, Wm[:128], W2[:128]], axis=1).astype(NPBF16)
    wsml = np.concatenate([Wm[128:], W2[128:]], axis=1).astype(NPBF16)
    bb = np.stack([b1[:128], bm[:128]], axis=1)
    bs = np.stack([b1[128:], bm[128:]], axis=1)
    b2p = b2[None, :].astype(NPBF16)

    shared = dict(wbig=np.ascontiguousarray(wbig),
                  wsml=np.ascontiguousarray(wsml),
                  bb=np.ascontiguousarray(bb), bs=np.ascontiguousarray(bs),
                  b2p=b2p, ones=np.ones((1, 128), NPBF16),
                  eye8=np.eye(NSEG, dtype=NPBF16))

    in_maps = []
    for c in range(ncores):
        rows = slice(c * rpc, (c + 1) * rpc)
        # [rpc, S, H] -> bf16 -> [128 tok-part, rpc, tch, H] flat
        fcore = feats[rows].astype(NPBF16)
        fpk = np.ascontiguousarray(
            fcore.reshape(rpc, tch, 128, H)
            .transpose(2, 0, 1, 3).reshape(128, rpc * tch * H))
        ohT = np.ascontiguousarray(
            oh16[rows].reshape(rpc, tch, 128, NSEG)
            .transpose(2, 0, 1, 3).reshape(128, rpc * tch * NSEG))
        icnt = np.ascontiguousarray(icnt_full[rows].T)
        in_maps.append(dict(features=fpk, ohT=ohT, icnt=icnt, **shared))
    return in_maps


def gather_output(core_outs, rpc=RPC):
    """[ngp*256, C] per core -> [8, 8, B, C]."""
    ncores = len(core_outs)
    ngp = rpc // 4
    full = np.empty((NSEG, NSEG, ncores * rpc, C), np.float32)
    for c, o in enumerate(core_outs):
        o = np.asarray(o).astype(np.float32)
        o = o.reshape(ngp, 4, NSEG, NSEG, C)          # gp, r4, i, j, c
        o = o.transpose(2, 3, 0, 1, 4).reshape(NSEG, NSEG, rpc, C)
        full[:, :, c * rpc:(c + 1) * rpc, :] = o
    return full


_NC_CACHE = {}


def _get_program():
    key = (RPC, TCH)
    if key not in _NC_CACHE:
        _NC_CACHE[key] = build_program()
    return _NC_CACHE[key]


def run(inputs, trace=False, trace_cores=None):
    nc = _get_program()
    in_maps = host_prep(**inputs)
    res = run_bass_kernel_spmd(
        nc, in_maps, core_ids=list(range(NCORES)),
        trace=trace, trace_cores=trace_cores,
    )
    out = gather_output([r["out"] for r in res.results])
    return out, res


def kernel(**inputs):
    out, _ = run(inputs, trace=False)
    return out
